# revision 57
# baseline (speedup 1.0000x reference)
"""
Trainium2 Bass kernel for nn_LinearLUT (residual-binarized LUT linear layer).

Math restructure
----------------
out[b,o] = bias[o] + sum_l sum_j Q_l[t, code_l[b,t]],  t=(o,j),
code_l[b,t] = sum_i 2^i * bit_l[b, idx_i(t)]  (4-bit code, ONE matmul/tile).

Fit each 16-entry LUT row Q_l[t,:] with an 11-parameter model in the code
bits: alpha + sum_i gamma_i b_i + sum_{i<k} c_ik b_i b_k, matched EXACTLY
at the 11 codes with <=2 bits set.  The residual R is nonzero only at the
5 codes {7,11,13,14,15}.

Device evaluation per level l:
  quadratic+linear: fold gamma onto the diagonal of a static per-(l,o)
    matrix M (bit^2 = bit), then
      Z_{l,o} = M_{l,o}^T @ bits_l          (PE, 32 matmuls)
      u = Z * bits_l                        (DVE, from PSUM)
      y_quad[o',(o,b)] += ones^T u          (PE; every row gets the column
                                             sum -> diagonal picks row o)
  constant:  rank-1 cvec stream (diag-compatible)
  residual:  5 one-hot planes (is_equal over the level-concatenated code
    tensor) streamed against tiny stationary Qres[j, o16] into the same
    [16, 2048] psum -- only the block diagonal o'==o is meaningful.

Host extracts res[b, o] = y[o, o*128+b] -- pure indexing -- and
concatenates cores (o is sharded 16/core).
"""

import numpy as np

import concourse.bass as bass
import concourse.bacc as bacc
import concourse.mybir as mybir
import concourse.tile as tile
from concourse.bass_utils import run_bass_kernel_spmd

# Problem dims (hardcoded per contract)
LEVELS = 2
K = 4
KK = 16
IN = 128
OUT = 128
B = 128
T = IN * OUT  # 16384
NCORES = 8
T_C = T // NCORES     # 2048 tables per core
OL = OUT // NCORES    # 16 out features per core
NTILE = T_C // 128    # 16 t-tiles per core

# codes with <=2 bits set are absorbed by the quadratic fit
C11 = [0, 1, 2, 4, 8, 3, 5, 6, 9, 10, 12]
VRES = [7, 11, 13, 14, 15]   # >=3 bits set: residual one-hot planes
NV = len(VRES)  # 5
PAIRS = [(0, 1), (0, 2), (0, 3), (1, 2), (1, 3), (2, 3)]

F16 = mybir.dt.float16
F32 = mybir.dt.float32

_CACHED_NC = None


def _build_nc():
    """Build the per-core Bass program (identical on all 8 cores)."""
    nc = bacc.Bacc("TRN2", target_bir_lowering=False, debug=False,
                   num_devices=NCORES)

    F8 = mybir.dt.float8e4
    xc = nc.dram_tensor("xc", [IN, B + 2], F32, kind="ExternalInput")
    g = nc.dram_tensor("g", [IN, T_C], F8, kind="ExternalInput")
    mq = nc.dram_tensor("mq", [128, LEVELS * OL * 128], F16,
                        kind="ExternalInput")
    qres = nc.dram_tensor("qres", [128, LEVELS * NV * OL], F16,
                          kind="ExternalInput")
    y = nc.dram_tensor("y", [OL, NTILE * B], F16, kind="ExternalOutput")

    NFILL = 8            # code-psum fills (2 tiles x 2 levels each)
    NCHUNK = 4           # 512-col chunks of the [16, 2048] output psum
    NWAVE = 8            # Z waves (4 (l,o) pairs each)

    with tile.TileContext(nc) as tc:
        with (
            tc.tile_pool(name="const", bufs=1) as cpool,
            tc.tile_pool(name="bits", bufs=1) as bpool,
            tc.tile_pool(name="codes", bufs=1) as cdpool,
            tc.tile_pool(name="eq", bufs=5) as eqpool,
            tc.tile_pool(name="u", bufs=1) as upool,
            tc.tile_pool(name="out", bufs=1) as opool,
            tc.tile_pool(name="psum_code", bufs=3,
                         space=bass.MemorySpace.PSUM) as pc,
            tc.tile_pool(name="psum_z", bufs=3,
                         space=bass.MemorySpace.PSUM) as pz,
            tc.tile_pool(name="psum_y", bufs=2,
                         space=bass.MemorySpace.PSUM) as py,
        ):
            xc_sb = cpool.tile([IN, B + 2], F32, tag="xc")
            xt_sb = xc_sb[:, 0:B]
            c_sb = xc_sb[:, B:B + 2]
            onesq_sb = cpool.tile([128, OL], F16, tag="onesq")
            qres_sb = cpool.tile([128, LEVELS * NV * OL], F16, tag="qres")
            # one dma_start rides one DMA engine (~50GB/s): split every big
            # tensor and spread across the three DMA-capable queues
            nc.gpsimd.memset(onesq_sb[:], 1.0)
            # startup-critical DMAs split small and spread: x halves on
            # scalar+sync first, g quarters next (gpsimd's land first and
            # feed fills 4-7), mq behind
            HX = 65
            nc.scalar.dma_start(xc_sb[:, HX:B + 2], xc[:, HX:B + 2])
            nc.sync.dma_start(xc_sb[:, 0:HX], xc[:, 0:HX])
            g_sb = cpool.tile([IN, T_C], F8, tag="g")
            nc.gpsimd.dma_start(g_sb[:, 1024:1536], g[:, 1024:1536])
            nc.gpsimd.dma_start(g_sb[:, 1536:2048], g[:, 1536:2048])
            nc.sync.dma_start(g_sb[:, 0:512], g[:, 0:512])
            nc.sync.dma_start(g_sb[:, 512:1024], g[:, 512:1024])
            nc.scalar.dma_start(qres_sb[:], qres[:])
            # mq chunk q feeds Z wave w=q: order queue positions so early
            # waves' chunks arrive first
            mq_sb = cpool.tile([128, LEVELS * OL * 128], F16, tag="mq")
            MCH = LEVELS * OL * 128 // 8
            mq_hosts = [nc.gpsimd, nc.scalar, nc.gpsimd, nc.scalar,
                        nc.gpsimd, nc.sync, nc.sync, nc.sync]
            for q in range(8):
                mq_hosts[q].dma_start(mq_sb[:, q * MCH:(q + 1) * MCH],
                                      mq[:, q * MCH:(q + 1) * MCH])

            # ---- sign bits (fp16 0/1, j on partitions), split so each
            # half starts as soon as its x-half lands; fp8 copy feeds the
            # fp8 code matmuls ----
            bits_cat = bpool.tile([IN, LEVELS * B], F16, tag="bits")
            bits8 = bpool.tile([IN, LEVELS * B], F8, tag="bits8")
            rc = bpool.tile([IN, B], F32, tag="rc")
            for (a, b_) in ((HX, B), (0, HX)):
                bit1 = bits_cat[:, a:b_]
                bit2 = bits_cat[:, B + a:B + b_]
                xs = xt_sb[:, a:b_]
                nc.vector.tensor_scalar(bit1, xs, 0.0, None,
                                        mybir.AluOpType.is_ge)
                nc.vector.scalar_tensor_tensor(
                    rc[:, a:b_], bit1, c_sb[:, 0:1], xs,
                    mybir.AluOpType.mult, mybir.AluOpType.add)
                nc.vector.tensor_scalar(bit2, rc[:, a:b_], c_sb[:, 1:2],
                                        None, mybir.AluOpType.is_ge)
            nc.vector.tensor_copy(bits8[:], bits_cat[:])
            # replicated bits for the qmult waves (4 o's per wave)
            brep = bpool.tile([IN, LEVELS * 512], F16, tag="brep")
            for l in range(LEVELS):
                for r in range(4):
                    nc.vector.tensor_copy(
                        brep[:, l * 512 + r * 128:l * 512 + r * 128 + 128],
                        bits_cat[:, l * B:(l + 1) * B])

            # ---- code matmuls + PSUM->SBUF drains; fills 4-7 first (their
            # g half lands first).  codes layout (half, l, tile8, b): each
            # half is CONTIGUOUS so its eq ops stay 2D and start as soon as
            # that half's four drains land ----
            codes = cdpool.tile([128, LEVELS * NTILE * B], F16, tag="codes")
            codes5 = codes[:].rearrange("p (h l t b) -> p h l t b", h=2,
                                        l=2, t=8)
            for f in [4, 5, 6, 7, 0, 1, 2, 3]:
                cps = pc.tile([128, 512], F32, tag="codepsum",
                              name=f"cps{f}")
                for dt_i in range(2):
                    t_i = 2 * f + dt_i
                    nc.tensor.matmul(
                        cps[:].rearrange("p (d l b) -> p d (l b)", d=2,
                                         l=2)[:, dt_i],
                        g_sb[:, t_i * 128:(t_i + 1) * 128],
                        bits8[:],
                        start=True, stop=True,
                    )
                src = cps[:].rearrange("p (d l b) -> p l d b", d=2, l=2)
                h = 0 if f >= 4 else 1
                t8 = (2 * f) % 8
                dst = codes5[:, h, :, t8:t8 + 2, :]
                if f in (4, 6):
                    nc.vector.tensor_copy(dst, src)
                else:
                    nc.scalar.copy(dst, src)

            # output chunks packed into 2 psum banks at partition
            # offsets 0/32/64 (base partition 96 is not allowed)
            y_bank0 = py.tile([128, 512], F32, tag="ypsum", name="yb0")
            y_bank1 = py.tile([128, 512], F32, tag="ypsum", name="yb1")
            y_ps = [y_bank0[0:OL, :], y_bank0[32:32 + OL, :],
                    y_bank0[64:64 + OL, :], y_bank1[0:OL, :]]

            # ---- quadratic forms: Z = M^T @ bits per (l, o); the first
            # four waves run now, waves 4-7 go after the LUT streams so
            # late mq chunks / psum recycling never stall the PE ----
            u_sb = upool.tile([128, LEVELS * OL * B], F16, tag="u")
            z_sb = upool.tile([128, LEVELS * OL * B], F16, tag="zsb")
            zps = [None] * NWAVE

            def emit_zwave(w):
                zw = pz.tile([128, 512], F32, tag="zpsum", name=f"z{w}")
                zps[w] = zw
                l = w // 4
                for zo in range(4):
                    o = (w % 4) * 4 + zo
                    mcol = (l * OL + o) * 128
                    nc.tensor.matmul(
                        zw[:, zo * 128:(zo + 1) * 128],
                        mq_sb[:, mcol:mcol + 128],
                        bits_cat[:, l * B:(l + 1) * B],
                        start=True, stop=True,
                    )
                # early waves: ACT moves Z to SBUF fp16 (emitted after all
                # code drains, so it cannot delay them) and the qmult runs
                # at the fast DVE fp16 rate; late waves skip the drain and
                # read PSUM directly, bypassing the saturated ACT chain
                nc.scalar.copy(z_sb[:, w * 512:(w + 1) * 512], zw[:])

            def emit_qmult(w):
                l = w // 4
                nc.vector.tensor_tensor(
                    u_sb[:, w * 512:(w + 1) * 512],
                    z_sb[:, w * 512:(w + 1) * 512],
                    brep[:, l * 512:(l + 1) * 512],
                    mybir.AluOpType.mult)

            for w in range(3):
                emit_zwave(w)

            # ---- one-hot residual half-planes + streamed contraction;
            # half 0 (tiles 8-15 -> chunks 2,3) drains first, so its eq
            # ops and LUT matmuls overlap the remaining fills/drains.
            # Z waves and qmults are staggered through the sequence ----
            ZW_AT = {2: [3], 4: [4], 5: [5], 6: [6], 7: [7]}
            QM_AT = {3: [0], 4: [1], 5: [2], 6: [3], 7: [4], 8: [5],
                     9: [6, 7]}
            for hi, h in enumerate((0, 1)):
                for vi, v in enumerate(VRES):
                    s = hi * NV + vi
                    eq = eqpool.tile([128, LEVELS * NTILE * B // 2], F16,
                                     tag="eq")
                    nc.vector.tensor_scalar(
                        eq[:], codes[:, h * 2048:(h + 1) * 2048], float(v),
                        None, mybir.AluOpType.is_equal)
                    for l in range(LEVELS):
                        qcol = (l * NV + vi) * OL
                        for cc in range(2):
                            c = (2 if h == 0 else 0) + cc
                            nc.tensor.matmul(
                                y_ps[c],
                                qres_sb[:, qcol:qcol + OL],
                                eq[:, l * 1024 + cc * 512:
                                   l * 1024 + cc * 512 + 512],
                                start=(vi == 0 and l == 0), stop=False,
                            )
                    for w in ZW_AT.get(s, []):
                        emit_zwave(w)
                    for w in QM_AT.get(s, []):
                        emit_qmult(w)

            # ---- u-sum + per-chunk drain & DMA ----
            y_sb = opool.tile([OL, NTILE * B], F16, tag="ysb")
            for c in range(NCHUNK):
                for l in range(LEVELS):
                    nc.tensor.matmul(
                        y_ps[c],
                        onesq_sb[:],
                        u_sb[:, l * 2048 + c * 512:l * 2048 + c * 512 + 512],
                        start=False, stop=(l == LEVELS - 1),
                    )
                dst = y_sb[:, c * 512:(c + 1) * 512]
                if c % 2 == 0:
                    nc.vector.tensor_copy(dst, y_ps[c])
                else:
                    nc.scalar.copy(dst, y_ps[c])
                nc.sync.dma_start(y[:, c * 512:(c + 1) * 512], dst)

    nc.compile()
    return nc


def _host_prep(x, weight, bias, means):
    """Weight-static preprocessing: per-level LUTs Q_l[t, v] (fp64)."""
    w = weight.astype(np.float64)
    m = np.abs(means.astype(np.float64))
    cc = np.arange(KK)
    tt = (2 * ((cc[:, None] >> np.arange(K)[None, :]) & 1) - 1).astype(
        np.float64)          # [c, i]
    sig = tt                  # same construction for sign patterns [v, i]

    qs = []
    for l in range(LEVELS):
        # M[v, c] = prod_i (1 + m_l * sig[v,i] * tt[c,i]) / 2
        M = np.prod((1.0 + m[l] * sig[:, None, :] * tt[None, :, :]) * 0.5,
                    axis=-1)  # [v, c]
        q = w @ M.T           # [T, KK]
        qs.append(q)
    return qs


def _feat(codes):
    """11-dim feature vector [1, b0..b3, pair products] per code."""
    codes = np.asarray(codes)
    b = ((codes[:, None] >> np.arange(K)[None, :]) & 1).astype(np.float64)
    cols = [np.ones(len(codes))] + [b[:, i] for i in range(K)]
    cols += [b[:, i] * b[:, k] for (i, k) in PAIRS]
    return np.stack(cols, axis=1)  # [n, 11]


def _quad_fit(qs):
    """Fit alpha/gamma/pair coefs exactly at C11; residual at VRES.

    Returns coef[l][T, 11] and resid[l][T, NV]."""
    A = _feat(C11)                 # [11, 11]
    Ainv = np.linalg.inv(A)
    Fres = _feat(VRES)             # [NV, 11]
    coefs, resid = [], []
    for l in range(LEVELS):
        c = qs[l][:, C11] @ Ainv.T          # [T, 11]
        r = qs[l][:, VRES] - c @ Fres.T     # [T, NV]
        coefs.append(c)
        resid.append(r)
    return coefs, resid


def _build_g(input_mask):
    G = np.zeros((IN, T), np.float64)
    cols = np.repeat(np.arange(T), K)
    vals = np.tile(2.0 ** np.arange(K), T)
    np.add.at(G, (input_mask.astype(np.int64), cols), vals)
    return G


def _build_m(input_mask, coefs, t0):
    """M[p, (l, o, q)] for this core's OL out-features starting at table t0.

    Quadratic-form matrices: diagonal gets gamma_i at idx_i; entry
    (idx_i, idx_k) accumulates the pair coefficient (single-sided; the
    device computes bits^T M bits so diagonal collisions are absorbed by
    bit^2 = bit)."""
    idx = input_mask.astype(np.int64).reshape(T, K)[t0:t0 + T_C]  # [T_C, 4]
    Ms = np.zeros((LEVELS, OL, 128, 128), np.float64)
    tloc = np.arange(T_C)
    o_of_t = tloc // IN
    for l in range(LEVELS):
        cf = coefs[l][t0:t0 + T_C]  # [T_C, 11]
        for i in range(K):
            np.add.at(Ms, (l, o_of_t, idx[:, i], idx[:, i]), cf[:, 1 + i])
        for pi, (i, k) in enumerate(PAIRS):
            np.add.at(Ms, (l, o_of_t, idx[:, i], idx[:, k]), cf[:, 5 + pi])
    # -> [p, (l, o, q)]
    return Ms.transpose(2, 0, 1, 3).reshape(128, -1)


def _make_in_maps(x, weight, bias, means, input_mask):
    qs = _host_prep(x, weight, bias, means)
    coefs, resid = _quad_fit(qs)
    G = _build_g(input_mask)

    m0 = float(np.abs(means.astype(np.float64))[0])
    xc = np.empty((IN, B + 2), np.float32)
    xc[:, :B] = x.astype(np.float32).T
    xc[:, B] = -2.0 * m0
    xc[:, B + 1] = -m0
    xc = np.ascontiguousarray(xc)

    # cvec[o] = bias[o] + sum_l sum_j alpha_l[o*IN+j]
    cvec_full = bias.astype(np.float64).copy()
    for l in range(LEVELS):
        cvec_full += coefs[l][:, 0].reshape(OUT, IN).sum(-1)

    import ml_dtypes
    in_maps = []
    for cid in range(NCORES):
        t0 = cid * T_C
        gc = G[:, t0:t0 + T_C].astype(ml_dtypes.float8_e4m3fn)
        mc = _build_m(input_mask, coefs, t0).astype(np.float16)
        # qres[j, (l, vi, o)] = resid_l[(o*IN+j), vi]
        qres_c = np.empty((128, LEVELS, NV, OL), np.float64)
        for l in range(LEVELS):
            rc = resid[l][t0:t0 + T_C].reshape(OL, IN, NV)
            qres_c[:, l, :, :] = rc.transpose(1, 2, 0)
        in_maps.append({
            "xc": xc,
            "g": np.ascontiguousarray(gc),
            "mq": np.ascontiguousarray(mc),
            "qres": np.ascontiguousarray(
                qres_c.reshape(128, -1).astype(np.float16)),
        })
    return in_maps, cvec_full


def kernel(x, weight, bias, means, input_mask):
    global _CACHED_NC
    if _CACHED_NC is None:
        _CACHED_NC = _build_nc()
    nc = _CACHED_NC

    in_maps, cvec_full = _make_in_maps(x, weight, bias, means, input_mask)
    res = run_bass_kernel_spmd(nc, in_maps, list(range(NCORES)))
    globals()["_LAST_RESULTS"] = res
    # y[o_local, tile*128 + b]: diagonal tile == o_local holds the result;
    # the per-o constant (bias + LUT fit constants) is added host-side
    out = np.empty((B, OUT), np.float32)
    for cid in range(NCORES):
        yc = res.results[cid]["y"].astype(np.float32)  # [OL, 2048]
        for o in range(OL):
            out[:, cid * OL + o] = yc[o, o * 128:(o + 1) * 128]
    out += cvec_full.astype(np.float32)[None, :]
    return out
